# revision 1
# baseline (speedup 1.0000x reference)
"""BitLinear 1.58 (nn_BitLinear158) Trainium2 Bass kernel.

Problem: x:[4,2048,4096] f32, weight:[4096,4096] f32 ->
         absmax-group-quantized x (8-bit fake quant, groups of 64) @
         ternary-quantized weight.T (per-row absmean scale) -> [4,2048,4096].

Sharding: data-parallel over tokens. Each of the 8 cores takes 1024 tokens
and the full weight; outputs concatenate along tokens. This replicates the
(cheap) weight pipeline but minimizes DMA+vector work versus sharding
out_features: x-quant is 8x smaller per core and HBM traffic per core is
96MiB vs 152MiB.

Per-core kernel:
  - activation quant: absmax per (token, 64-group), scale=127*recip(absmax),
    q=rint(x*scale) via the +/-1.5*2^23 trick (round-half-even, matching
    jnp.round), x_q=q*(absmax/127) cast to fp16, transposed on the fly
    (xbar dma transpose) into the matmul-stationary layout [128,K/128,M].
  - weight ternarize: s=max(mean|row|,eps) with a two-stage compensated
    reduction (group sums, then an exact 2^-12-grid split so the final
    accumulation is error-free: my s is closer to the true mean than any
    f32 single-pass sum, minimizing disagreement with the f32 reference
    at the discontinuous round(w/s) boundaries); ternary values computed
    as exact comparisons t = (w > 0.5s) - (w < -0.5s), which equals
    clip(round-half-even(w/s),-1,1) for all non-boundary w and avoids any
    divide/round rounding concerns. t is exact in fp16.
  - matmul: psum[m,o] += xq_t[:,ks,m-block].T @ t_t[:,ks,o-tile] over the
    32 contraction chunks, fp16 operands, fp32 psum.
  - eviction: out = psum * s_row (per-column broadcast of s).
"""
import sys

sys.path.insert(0, "/opt/trn_rl_repo")

import numpy as np

B, S, D_IN, D_OUT = 4, 2048, 4096, 4096
N_CORES = 8
M_TOT = B * S
M_C = M_TOT // N_CORES

P = 128
G = 64
OT = 256                        # columns per o-tile (psum free dim)
MAGIC = float(1.5 * 2.0 ** 23)  # fp32 round-to-nearest-even trick
MAGIC2 = float(1.5 * 2.0 ** 11)  # quantize-to-2^-12-grid trick
EPS = 1e-5
QMAX = 127.0
INV_QMAX = float(np.float32(1.0 / 127.0))

_cache = {}


def _build(M, K, O):
    import concourse.bass as bass
    import concourse.tile as tile
    from concourse import bacc, mybir

    f32 = mybir.dt.float32
    f16 = mybir.dt.float16
    Alu = mybir.AluOpType
    Ax = mybir.AxisListType

    KSUB = K // P
    MB = M // P
    NOT = O // OT
    OSUB = OT // P
    NG = K // G

    nc = bacc.Bacc("TRN2", target_bir_lowering=False, num_devices=1)
    x = nc.dram_tensor("x", [M, K], f32, kind="ExternalInput")
    w = nc.dram_tensor("w", [O, K], f32, kind="ExternalInput")
    out = nc.dram_tensor("out", [M, O], f32, kind="ExternalOutput")
    s_scr = nc.dram_tensor("s_scr", [O, 1], f32, kind="Internal")

    xap, wap, oap = x.ap(), w.ap(), out.ap()

    with tile.TileContext(nc) as tc:
        with (
            tc.tile_pool(name="xq", bufs=1) as xq_pool,
            tc.tile_pool(name="stage", bufs=3) as stage,
            tc.tile_pool(name="f16w", bufs=2) as f16w,
            tc.tile_pool(name="tt", bufs=2) as tt_pool,
            tc.tile_pool(name="small", bufs=4) as small,
            tc.tile_pool(name="ev", bufs=4) as ev_pool,
            tc.tile_pool(name="sb", bufs=2) as sb_pool,
            tc.tile_pool(name="ps", bufs=8, space="PSUM") as ps_pool,
        ):
            # -------- activation quantization + transpose --------
            xq_t = xq_pool.tile([P, KSUB, M], f16)
            for mb in range(MB):
                xt = stage.tile([P, K], f32, tag="stage")
                nc.sync.dma_start(xt[:], xap[mb * P:(mb + 1) * P, :])
                xg = xt.rearrange("p (g e) -> p g e", e=G)
                am = small.tile([P, NG], f32, tag="am")
                nc.vector.tensor_reduce(am[:], xg, Ax.X, Alu.max,
                                        apply_absolute_value=True)
                am2 = small.tile([P, NG], f32, tag="am2")
                nc.vector.tensor_scalar(am2[:], am[:], EPS, None, Alu.max)
                rc = small.tile([P, NG], f32, tag="rc")
                nc.vector.reciprocal(rc[:], am2[:])
                scale = small.tile([P, NG], f32, tag="scale")
                nc.vector.tensor_scalar(scale[:], rc[:], QMAX, None, Alu.mult)
                inv = small.tile([P, NG], f32, tag="inv")
                nc.vector.tensor_scalar(inv[:], am2[:], INV_QMAX, None,
                                        Alu.mult)
                nc.vector.tensor_tensor(
                    xg, xg, scale[:, :, None].to_broadcast((P, NG, G)),
                    Alu.mult)
                nc.vector.tensor_scalar(xt[:], xt[:], MAGIC, MAGIC,
                                        Alu.add, Alu.subtract)
                xq16 = f16w.tile([P, K], f16, tag="cmp_p")
                nc.vector.tensor_tensor(
                    xq16.rearrange("p (g e) -> p g e", e=G), xg,
                    inv[:, :, None].to_broadcast((P, NG, G)), Alu.mult)
                nc.sync.dma_start_transpose(
                    xq_t[:, :, mb * P:(mb + 1) * P], xq16[:])

            # -------- per-o-tile: ternarize weight rows + matmul --------
            for ot in range(NOT):
                tt = tt_pool.tile([P, KSUB, OT], f16)
                for osub in range(OSUB):
                    o0 = ot * OT + osub * P
                    wt = stage.tile([P, K], f32, tag="stage")
                    nc.sync.dma_start(wt[:], wap[o0:o0 + P, :])
                    wg = wt.rearrange("p (g e) -> p g e", e=G)
                    gs = small.tile([P, K // G], f32, tag="gs")
                    nc.vector.tensor_reduce(gs[:], wg, Ax.X, Alu.add,
                                            apply_absolute_value=True)
                    hq = small.tile([P, K // G], f32, tag="hq")
                    nc.vector.tensor_scalar(hq[:], gs[:], MAGIC2, MAGIC2,
                                            Alu.add, Alu.subtract)
                    lq = small.tile([P, K // G], f32, tag="lq")
                    nc.vector.tensor_tensor(lq[:], gs[:], hq[:], Alu.subtract)
                    sh = small.tile([P, 1], f32, tag="sh")
                    nc.vector.tensor_reduce(sh[:], hq[:], Ax.X, Alu.add)
                    sl = small.tile([P, 1], f32, tag="sl")
                    nc.vector.tensor_reduce(sl[:], lq[:], Ax.X, Alu.add)
                    ssum = small.tile([P, 1], f32, tag="ssum")
                    nc.vector.tensor_tensor(ssum[:], sh[:], sl[:], Alu.add)
                    sv = small.tile([P, 1], f32, tag="sv")
                    nc.vector.tensor_scalar(sv[:], ssum[:],
                                            float(np.float32(1.0 / K)),
                                            EPS, Alu.mult, Alu.max)
                    nc.sync.dma_start(s_scr.ap()[o0:o0 + P, :], sv[:])
                    bp = small.tile([P, 1], f32, tag="bp")
                    nc.vector.tensor_scalar(bp[:], sv[:], 0.5, None, Alu.mult)
                    bn = small.tile([P, 1], f32, tag="bn")
                    nc.vector.tensor_scalar(bn[:], sv[:], -0.5, None, Alu.mult)
                    cp = f16w.tile([P, K], f16, tag="cmp_p")
                    nc.vector.tensor_scalar(cp[:], wt[:], bp[:], None,
                                            Alu.is_gt)
                    cn = f16w.tile([P, K], f16, tag="cmp_n")
                    nc.gpsimd.tensor_scalar(cn[:], wt[:], bn[:], None,
                                            Alu.is_lt)
                    nc.vector.tensor_tensor(cp[:], cp[:], cn[:], Alu.subtract)
                    nc.sync.dma_start_transpose(
                        tt[:, :, osub * P:(osub + 1) * P], cp[:])

                s_base = s_scr.ap()[ot * OT:(ot + 1) * OT, 0]
                s_bc_ap = bass.AP(tensor=s_base.tensor, offset=s_base.offset,
                                  ap=[[0, P], *s_base.ap])
                sbc = sb_pool.tile([P, OT], f32)
                nc.gpsimd.dma_start(sbc[:], s_bc_ap)

                for mb in range(MB):
                    ps = ps_pool.tile([P, OT], f32)
                    for ks in range(KSUB):
                        nc.tensor.matmul(
                            ps[:], xq_t[:, ks, mb * P:(mb + 1) * P],
                            tt[:, ks, :],
                            start=(ks == 0), stop=(ks == KSUB - 1))
                    ev = ev_pool.tile([P, OT], f32)
                    nc.vector.tensor_tensor(ev[:], ps[:], sbc[:], Alu.mult)
                    nc.sync.dma_start(
                        oap[mb * P:(mb + 1) * P, ot * OT:(ot + 1) * OT],
                        ev[:])

    nc.compile()
    return nc


def _get_nc():
    if "nc" not in _cache:
        _cache["nc"] = _build(M_C, D_IN, D_OUT)
    return _cache["nc"]


def run(x, weight, trace=False):
    """Run on 8 NeuronCores; returns (full output [B,S,D_OUT], results obj)."""
    from concourse.bass_utils import run_bass_kernel_spmd

    x = np.ascontiguousarray(np.asarray(x, dtype=np.float32))
    w = np.ascontiguousarray(np.asarray(weight, dtype=np.float32))
    assert x.shape == (B, S, D_IN) and w.shape == (D_OUT, D_IN)
    xf = x.reshape(M_TOT, D_IN)
    nc = _get_nc()
    in_maps = [
        {"x": np.ascontiguousarray(xf[c * M_C:(c + 1) * M_C]), "w": w}
        for c in range(N_CORES)
    ]
    res = run_bass_kernel_spmd(nc, in_maps, core_ids=list(range(N_CORES)),
                               trace=trace)
    outf = np.concatenate([res.results[c]["out"] for c in range(N_CORES)],
                          axis=0)
    return outf.reshape(B, S, D_OUT), res


def kernel(x, weight):
    out, _ = run(x, weight)
    return out


# revision 2
# speedup vs baseline: 2.6130x; 2.6130x over previous
"""BitLinear 1.58 (nn_BitLinear158) Trainium2 Bass kernel.

Problem: x:[4,2048,4096] f32, weight:[4096,4096] f32 ->
         absmax-group-quantized x (8-bit fake quant, groups of 64) @
         ternary-quantized weight.T (per-row absmean scale) -> [4,2048,4096].

Sharding: data-parallel over tokens. Each of the 8 cores takes 1024 tokens
and the full weight; outputs concatenate along tokens. This replicates the
(cheap) weight pipeline but minimizes DMA+vector work versus sharding
out_features: x-quant is 8x smaller per core and HBM traffic per core is
96MiB vs 152MiB.

Per-core kernel:
  - activation quant: absmax per (token, 64-group), scale=127*recip(absmax),
    q=rint(x*scale) via the +/-1.5*2^23 trick (round-half-even, matching
    jnp.round), x_q=q*(absmax/127) cast to fp16, transposed on the fly
    (xbar dma transpose) into the matmul-stationary layout [128,K/128,M].
  - weight ternarize: s=max(mean|row|,eps) with a two-stage compensated
    reduction (group sums, then an exact 2^-12-grid split so the final
    accumulation is error-free: my s is closer to the true mean than any
    f32 single-pass sum, minimizing disagreement with the f32 reference
    at the discontinuous round(w/s) boundaries); ternary values computed
    as exact comparisons t = (w > 0.5s) - (w < -0.5s), which equals
    clip(round-half-even(w/s),-1,1) for all non-boundary w and avoids any
    divide/round rounding concerns. t is exact in fp16.
  - matmul: psum[m,o] += xq_t[:,ks,m-block].T @ t_t[:,ks,o-tile] over the
    32 contraction chunks, fp16 operands, fp32 psum.
  - eviction: out = psum * s_row (per-column broadcast of s).
"""
import sys

sys.path.insert(0, "/opt/trn_rl_repo")

import numpy as np

B, S, D_IN, D_OUT = 4, 2048, 4096, 4096
N_CORES = 8
M_TOT = B * S
M_C = M_TOT // N_CORES

P = 128
G = 64
OT = 256                        # columns per o-tile (psum free dim)
MAGIC = float(1.5 * 2.0 ** 23)  # fp32 round-to-nearest-even trick
MAGIC2 = float(1.5 * 2.0 ** 11)  # quantize-to-2^-12-grid trick
EPS = 1e-5
QMAX = 127.0
INV_QMAX = float(np.float32(1.0 / 127.0))

_cache = {}


def _build(M, K, O):
    import concourse.bass as bass
    import concourse.tile as tile
    from concourse import bacc, mybir

    f32 = mybir.dt.float32
    f16 = mybir.dt.float16
    Alu = mybir.AluOpType
    Act = mybir.ActivationFunctionType
    Ax = mybir.AxisListType

    KSUB = K // P
    MB = M // P
    NOT = O // OT
    OSUB = OT // P
    NG = K // G

    nc = bacc.Bacc("TRN2", target_bir_lowering=False, num_devices=1)
    x = nc.dram_tensor("x", [M, K], f32, kind="ExternalInput")
    w = nc.dram_tensor("w", [O, K], f32, kind="ExternalInput")
    out = nc.dram_tensor("out", [M, O], f32, kind="ExternalOutput")
    s_scr = nc.dram_tensor("s_scr", [O, 1], f32, kind="Internal")

    xap, wap, oap = x.ap(), w.ap(), out.ap()

    with tile.TileContext(nc) as tc:
        with (
            tc.tile_pool(name="xq", bufs=1) as xq_pool,
            tc.tile_pool(name="stage", bufs=3) as stage,
            tc.tile_pool(name="f16w", bufs=2) as f16w,
            tc.tile_pool(name="tt", bufs=2) as tt_pool,
            tc.tile_pool(name="small", bufs=4) as small,
            tc.tile_pool(name="ev", bufs=4) as ev_pool,
            tc.tile_pool(name="sb", bufs=2) as sb_pool,
            tc.tile_pool(name="ps", bufs=8, space="PSUM") as ps_pool,
        ):
            # -------- activation quantization + transpose --------
            xq_t = xq_pool.tile([P, KSUB, M], f16)
            for mb in range(MB):
                xt = stage.tile([P, K], f32, tag="stage")
                nc.sync.dma_start(xt[:], xap[mb * P:(mb + 1) * P, :])
                xg = xt.rearrange("p (g e) -> p g e", e=G)
                am = small.tile([P, NG], f32, tag="am")
                nc.vector.tensor_reduce(am[:], xg, Ax.X, Alu.max,
                                        apply_absolute_value=True)
                am2 = small.tile([P, NG], f32, tag="am2")
                nc.vector.tensor_scalar(am2[:], am[:], EPS, None, Alu.max)
                rc = small.tile([P, NG], f32, tag="rc")
                nc.vector.reciprocal(rc[:], am2[:])
                scale = small.tile([P, NG], f32, tag="scale")
                nc.vector.tensor_scalar(scale[:], rc[:], QMAX, None, Alu.mult)
                inv = small.tile([P, NG], f32, tag="inv")
                nc.vector.tensor_scalar(inv[:], am2[:], INV_QMAX, None,
                                        Alu.mult)
                nc.vector.tensor_tensor(
                    xg, xg, scale[:, :, None].to_broadcast((P, NG, G)),
                    Alu.mult)
                nc.vector.tensor_scalar(xt[:], xt[:], MAGIC, MAGIC,
                                        Alu.add, Alu.subtract)
                xq16 = f16w.tile([P, K], f16, tag="cmp_p")
                nc.vector.tensor_tensor(
                    xq16.rearrange("p (g e) -> p g e", e=G), xg,
                    inv[:, :, None].to_broadcast((P, NG, G)), Alu.mult)
                nc.sync.dma_start_transpose(
                    xq_t[:, :, mb * P:(mb + 1) * P], xq16[:])

            # -------- per-o-tile: ternarize weight rows + matmul --------
            for ot in range(NOT):
                tt = tt_pool.tile([P, KSUB, OT], f16)
                for osub in range(OSUB):
                    o0 = ot * OT + osub * P
                    wt = stage.tile([P, K], f32, tag="stage")
                    nc.sync.dma_start(wt[:], wap[o0:o0 + P, :])
                    wg = wt.rearrange("p (g e) -> p g e", e=G)
                    gs = small.tile([P, K // G], f32, tag="gs")
                    nc.vector.tensor_reduce(gs[:], wg, Ax.X, Alu.add,
                                            apply_absolute_value=True)
                    hq = small.tile([P, K // G], f32, tag="hq")
                    nc.vector.tensor_scalar(hq[:], gs[:], MAGIC2, MAGIC2,
                                            Alu.add, Alu.subtract)
                    lq = small.tile([P, K // G], f32, tag="lq")
                    nc.vector.tensor_tensor(lq[:], gs[:], hq[:], Alu.subtract)
                    sh = small.tile([P, 1], f32, tag="sh")
                    nc.vector.tensor_reduce(sh[:], hq[:], Ax.X, Alu.add)
                    sl = small.tile([P, 1], f32, tag="sl")
                    nc.vector.tensor_reduce(sl[:], lq[:], Ax.X, Alu.add)
                    ssum = small.tile([P, 1], f32, tag="ssum")
                    nc.vector.tensor_tensor(ssum[:], sh[:], sl[:], Alu.add)
                    sv = small.tile([P, 1], f32, tag="sv")
                    nc.vector.tensor_scalar(sv[:], ssum[:],
                                            float(np.float32(1.0 / K)),
                                            EPS, Alu.mult, Alu.max)
                    bp = small.tile([P, 1], f32, tag="bp")
                    nc.vector.tensor_scalar(bp[:], sv[:], 0.5, None, Alu.mult)
                    bn = small.tile([P, 1], f32, tag="bn")
                    nc.vector.tensor_scalar(bn[:], sv[:], -0.5, None, Alu.mult)
                    # eviction scale is 0.5*s (the sign-sum below is 2t)
                    nc.sync.dma_start(s_scr.ap()[o0:o0 + P, :], bp[:])
                    # 2t = sign(w - 0.5s) + sign(w + 0.5s); the sign of an
                    # f32 subtract is exact, so this equals the exact
                    # comparison ternarization (2x) everywhere off-boundary
                    sga = f16w.tile([P, K], f16, tag="cmp_p")
                    nc.scalar.activation(out=sga[:], in_=wt[:], func=Act.Sign,
                                         bias=bn[:], scale=1.0)
                    sgb = f16w.tile([P, K], f16, tag="cmp_n")
                    nc.scalar.activation(out=sgb[:], in_=wt[:], func=Act.Sign,
                                         bias=bp[:], scale=1.0)
                    nc.vector.tensor_tensor(sga[:], sga[:], sgb[:], Alu.add)
                    nc.sync.dma_start_transpose(
                        tt[:, :, osub * P:(osub + 1) * P], sga[:])

                s_base = s_scr.ap()[ot * OT:(ot + 1) * OT, 0]
                s_bc_ap = bass.AP(tensor=s_base.tensor, offset=s_base.offset,
                                  ap=[[0, P], *s_base.ap])
                sbc = sb_pool.tile([P, OT], f32)
                nc.gpsimd.dma_start(sbc[:], s_bc_ap)

                for mb in range(MB):
                    ps = ps_pool.tile([P, OT], f32)
                    for ks in range(KSUB):
                        nc.tensor.matmul(
                            ps[:], xq_t[:, ks, mb * P:(mb + 1) * P],
                            tt[:, ks, :],
                            start=(ks == 0), stop=(ks == KSUB - 1))
                    ev = ev_pool.tile([P, OT], f32)
                    nc.vector.tensor_tensor(ev[:], ps[:], sbc[:], Alu.mult)
                    nc.sync.dma_start(
                        oap[mb * P:(mb + 1) * P, ot * OT:(ot + 1) * OT],
                        ev[:])

    nc.compile()
    return nc


def _get_nc():
    if "nc" not in _cache:
        _cache["nc"] = _build(M_C, D_IN, D_OUT)
    return _cache["nc"]


def run(x, weight, trace=False):
    """Run on 8 NeuronCores; returns (full output [B,S,D_OUT], results obj)."""
    from concourse.bass_utils import run_bass_kernel_spmd

    x = np.ascontiguousarray(np.asarray(x, dtype=np.float32))
    w = np.ascontiguousarray(np.asarray(weight, dtype=np.float32))
    assert x.shape == (B, S, D_IN) and w.shape == (D_OUT, D_IN)
    xf = x.reshape(M_TOT, D_IN)
    nc = _get_nc()
    in_maps = [
        {"x": np.ascontiguousarray(xf[c * M_C:(c + 1) * M_C]), "w": w}
        for c in range(N_CORES)
    ]
    res = run_bass_kernel_spmd(nc, in_maps, core_ids=list(range(N_CORES)),
                               trace=trace)
    outf = np.concatenate([res.results[c]["out"] for c in range(N_CORES)],
                          axis=0)
    return outf.reshape(B, S, D_OUT), res


def kernel(x, weight):
    out, _ = run(x, weight)
    return out
